# revision 30
# baseline (speedup 1.0000x reference)
"""Trainium2 Bass kernel for the Actor moe_routing module.

Computes, on 8 NeuronCores (tensor-parallel over the fragment dim N):
    h        = relu(cond @ W1 + b1)                      [B, H]
    logits   = where(mask, -inf, h @ W2 + b2)            [B, N]
    idx      = argmax(logits + gumbel, axis=1)           [B]
    onehot   = one_hot(idx, N)                           [B, N]  (== gumbel-softmax hard forward)
    fragment = frag_table[idx]                           [B, D]
    merger   = relu(cond @ Wa + fragment @ Wb + bm)      [B, D]

Sharding: each core owns a 2048-column shard of N (W2/b2/mask/gumbel column
shards, logits written per-shard); the argmax is combined across cores with a
tiny AllGather; onehot/fragment/merger are produced B-sharded (512 rows per
core) using indirect DMA with out-of-bounds predication.

MODE:
  "fp32" - full-precision matmuls for h/logits (4 cyc/row).
  "bf3"  - 3-term bf16 split (hi*hi + hi*lo + lo*hi) for the logits matmul
           (3 cyc/row, ~2^-17 rel err; verified 0 argmax flips on the
           reference inputs).
"""
import sys
sys.path.insert(0, '/opt/trn_rl_repo')

import numpy as np
import concourse.bass as bass
import concourse.mybir as mybir
import concourse.tile as tile
from concourse import bacc
from concourse.bass_utils import run_bass_kernel_spmd
from concourse.masks import make_identity

f32 = mybir.dt.float32
bf16 = mybir.dt.bfloat16
i32 = mybir.dt.int32
u32 = mybir.dt.uint32
u8 = mybir.dt.uint8
OP = mybir.AluOpType
AF = mybir.ActivationFunctionType

B, N, D, H = 4096, 16384, 256, 1024
NC = 8
NS = N // NC          # 2048 n-columns per core
BS = B // NC          # 512 b-rows per core (output sharding)
NBT = 8               # B-tiles of 512 rows
NBC = B // 128        # 32 row-chunks of 128
BIGNEG = -3.4e38      # x + BIGNEG + BIGNEG == -inf for |x| small
OOB = 1 << 22

MODE = "bf3"

_CACHE = {}


def _build(mode, with_b2, with_bm):
    import os
    phase = int(os.environ.get("KPH", "3"))
    nc = bacc.Bacc("TRN2", target_bir_lowering=False, debug=False, num_devices=NC)

    cond = nc.dram_tensor("cond", [B, D], f32, kind="ExternalInput").ap()
    maskb = nc.dram_tensor("maskb", [B, NS], u8, kind="ExternalInput").ap()
    gumb = nc.dram_tensor("gumb", [B, NS], f32, kind="ExternalInput").ap()
    w1 = nc.dram_tensor("w1", [D, H], f32, kind="ExternalInput").ap()
    b1 = nc.dram_tensor("b1", [128, H // 128], f32, kind="ExternalInput").ap()
    w2h = nc.dram_tensor("w2h", [H, NS], bf16, kind="ExternalInput").ap()
    w2l = nc.dram_tensor("w2l", [H, NS], bf16, kind="ExternalInput").ap()
    b2 = nc.dram_tensor("b2", [1, NS], f32, kind="ExternalInput").ap()
    frag = nc.dram_tensor("frag", [N, D], f32, kind="ExternalInput").ap()
    wa = nc.dram_tensor("wa", [D, D], f32, kind="ExternalInput").ap()
    wb = nc.dram_tensor("wb", [D, D], f32, kind="ExternalInput").ap()
    bm = nc.dram_tensor("bm", [1, D], f32, kind="ExternalInput").ap()

    logits_o = nc.dram_tensor("logits_o", [B, NS], f32, kind="ExternalOutput").ap()
    onehot_o = nc.dram_tensor("onehot_o", [BS, N], f32, kind="ExternalOutput").ap()
    frag_o = nc.dram_tensor("frag_o", [BS, D], f32, kind="ExternalOutput").ap()
    merg_o = nc.dram_tensor("merg_o", [BS, D], f32, kind="ExternalOutput").ap()


    with tile.TileContext(nc) as tc:
        with tc.tile_pool(name="const", bufs=1) as cp:
            ident = cp.tile([128, 128], f32)
            make_identity(nc, ident[:])
            identb = cp.tile([128, 128], bf16)
            nc.vector.tensor_copy(out=identb[:], in_=ident[:])
            ones1 = cp.tile([1, 128], f32)
            nc.gpsimd.memset(ones1[:], 1.0)
            # iota127[p, j] = j  (compare target for one-hot remainders)
            iota127i = cp.tile([128, 128], i32)
            nc.gpsimd.iota(iota127i[:], pattern=[[1, 128]], base=0, channel_multiplier=0)
            iota127 = cp.tile([128, 128], f32)
            nc.vector.tensor_copy(out=iota127[:], in_=iota127i[:])
            # biota[p, c] = 128*c + p  (global row index per (chunk, partition))
            biotai = cp.tile([128, NBC], i32)
            nc.gpsimd.iota(biotai[:], pattern=[[128, NBC]], base=0, channel_multiplier=1)
            # local (max | argmax) for all 32 row-chunks, AllGather payload layout
            lboth = cp.tile([128, 64], f32)
            b1sb = cp.tile([128, H // 128], f32)
            nc.sync.dma_start(out=b1sb[:], in_=b1[:])
            b2sb = cp.tile([1, NS], f32)
            nc.sync.dma_start(out=b2sb[:], in_=b2[:])
            bmsb = cp.tile([1, D], f32)
            nc.sync.dma_start(out=bmsb[:], in_=bm[:])
            # partition id -> f32 broadcast column
            pid_sb = cp.tile([1, 1], u32)
            nc.sync.dma_start(out=pid_sb[:], in_=nc.partition_id_tensor.ap()[:])
            pid1f = cp.tile([1, 1], f32)
            nc.vector.tensor_copy(out=pid1f[:], in_=pid_sb[:])
            pidc = cp.tile([128, 1], f32)

            with tc.tile_pool(name="psc", bufs=1, space="PSUM") as psc:
                pp = psc.tile([128, 1], f32)
                nc.tensor.matmul(pp[:], ones1[:], pid1f[:], start=True, stop=True)
                nc.scalar.copy(pidc[:], pp[:])

            # W1 resident [128, 2, H] (k-chunk major)
            w1sb = cp.tile([128, 2, H], f32)
            for kc in range(2):
                nc.sync.dma_start(out=w1sb[:, kc, :], in_=w1[kc * 128:(kc + 1) * 128, :])
            wasb = cp.tile([128, 2, D], bf16)
            wbsb = cp.tile([128, 2, D], bf16)
            wtmp = cp.tile([128, 2, D], f32)
            for kc in range(2):
                nc.sync.dma_start(out=wtmp[:, kc, :], in_=wa[kc * 128:(kc + 1) * 128, :])
            nc.vector.tensor_copy(out=wasb[:], in_=wtmp[:])
            wtmp2 = cp.tile([128, 2, D], f32)
            for kc in range(2):
                nc.sync.dma_start(out=wtmp2[:, kc, :], in_=wb[kc * 128:(kc + 1) * 128, :])
            nc.vector.tensor_copy(out=wbsb[:], in_=wtmp2[:])

            # ---------------- main loop + split collectives ----------------
            # B is processed in two halves; each half's argmax AllGather, combine
            # and output phase interleave under the other half's matmul stream.
            with tc.tile_pool(name="w2p", bufs=1) as w2p, \
                 tc.tile_pool(name="cc", bufs=1) as ccp, \
                 tc.tile_pool(name="ccd", bufs=1, space="DRAM") as ccd, \
                 tc.tile_pool(name="oc", bufs=1) as ocp, \
                 tc.tile_pool(name="bt", bufs=2) as btp, \
                 tc.tile_pool(name="bc", bufs=2) as bcp, \
                 tc.tile_pool(name="lo", bufs=2) as lop, \
                 tc.tile_pool(name="ps", bufs=3, space="PSUM") as psp, \
                 tc.tile_pool(name="psh", bufs=2, space="PSUM") as psh:
                if mode == "fp32":
                    raise NotImplementedError("fp32 mode disabled after host-side W2 split")
                w2hi = w2p.tile([128, 8, NS], bf16)
                w2lo = w2p.tile([128, 8, NS], bf16)
                SPLITS = [(0, 12), (12, 12), (24, 4), (28, 4)]
                cc_in = [ccd.tile([128, 2 * w], f32, name=f"cci{h}")
                         for h, (_, w) in enumerate(SPLITS)]
                cc_out = [ccd.tile([NC * 128, 2 * w], f32, addr_space="Shared", name=f"cco{h}")
                          for h, (_, w) in enumerate(SPLITS)]
                gidx = ccp.tile([128, NBC], f32)
                gint_a = ccp.tile([128, NBC], i32)
                blocint_a = ccp.tile([128, NBC], i32)
                segint_a = ccp.tile([128, NBC], i32)
                rf_a = ccp.tile([128, NBC], f32)
                pid512 = ccp.tile([128, 1], f32)
                nc.vector.tensor_scalar(pid512[:], pidc[:], float(BS), None, OP.mult)
                onehot_v = onehot_o.rearrange("b (q r) -> (b q) r", r=128)

                def emit_pack(h):
                    st, w = SPLITS[h]
                    # payload [128, 2w]: (lmax cols | lidx cols) for split h
                    nc.gpsimd.dma_start(out=cc_in[h][:, 0:w], in_=lboth[:, st:st + w])
                    nc.gpsimd.dma_start(out=cc_in[h][:, w:2 * w],
                                        in_=lboth[:, 32 + st:32 + st + w])
                    nc.gpsimd.collective_compute(
                        "AllGather", OP.bypass,
                        replica_groups=[list(range(NC))],
                        ins=[cc_in[h].opt()], outs=[cc_out[h].opt()],
                    )

                def emit_combine(h):
                    st, w = SPLITS[h]
                    bsl = slice(st, st + w)
                    valsidx = ccp.tile([128, NC, 2 * w], f32, name=f"vi{h}")
                    for r in range(NC):
                        nc.sync.dma_start(out=valsidx[:, r, :],
                                          in_=cc_out[h][r * 128:(r + 1) * 128, :])
                    gmax = ccp.tile([128, w], f32, name=f"gmax{h}")
                    nc.vector.tensor_copy(out=gmax[:], in_=valsidx[:, 0, 0:w])
                    for r in range(1, NC):
                        nc.vector.tensor_tensor(out=gmax[:], in0=gmax[:],
                                                in1=valsidx[:, r, 0:w], op=OP.max)
                    taken = ccp.tile([128, w], f32, name=f"tk{h}")
                    nc.vector.memset(taken[:], 0.0)
                    nc.vector.memset(gidx[:, bsl], 0.0)
                    wt = ccp.tile([128, w], f32, name=f"wt{h}")
                    omt = ccp.tile([128, w], f32, name=f"omt{h}")
                    term = ccp.tile([128, w], f32, name=f"tm{h}")
                    for r in range(NC):
                        nc.vector.tensor_tensor(out=wt[:], in0=valsidx[:, r, 0:w],
                                                in1=gmax[:], op=OP.is_equal)
                        nc.vector.tensor_scalar(omt[:], taken[:], -1.0, 1.0, OP.mult, OP.add)
                        nc.vector.tensor_tensor(out=wt[:], in0=wt[:], in1=omt[:], op=OP.mult)
                        nc.vector.tensor_scalar(term[:], valsidx[:, r, w:2 * w], float(r * NS),
                                                None, OP.add)
                        nc.vector.tensor_tensor(out=term[:], in0=term[:], in1=wt[:], op=OP.mult)
                        nc.vector.tensor_tensor(out=gidx[:, bsl], in0=gidx[:, bsl],
                                                in1=term[:], op=OP.add)
                        nc.vector.tensor_tensor(out=taken[:], in0=taken[:], in1=wt[:], op=OP.max)

                def emit_prep(h):
                    st, w = SPLITS[h]
                    # per-group index prep for split h (all DVE)
                    for g in range(st // 4, (st + w) // 4):
                        bsl = slice(4 * g, 4 * g + 4)
                        mine = ccp.tile([128, 1], f32, name=f"mine{g}")
                        nc.vector.tensor_scalar(mine[:], pidc[:], float(g), None, OP.is_equal)
                        nmbig = ccp.tile([128, 1], f32, name=f"nmbig{g}")
                        nc.vector.tensor_scalar(nmbig[:], mine[:], float(-OOB), float(OOB),
                                                OP.mult, OP.add)
                        gadj = ccp.tile([128, 4], f32, name=f"gadj{g}")
                        nc.vector.tensor_scalar(gadj[:], gidx[:, bsl], nmbig[:, :1], None, OP.add)
                        nc.vector.tensor_copy(out=gint_a[:, bsl], in_=gadj[:])
                        blocf = ccp.tile([128, 4], f32, name=f"blocf{g}")
                        nc.vector.tensor_copy(out=blocf[:], in_=biotai[:, bsl])
                        nc.vector.tensor_scalar(blocf[:], blocf[:], pid512[:, :1], None, OP.subtract)
                        nc.vector.tensor_scalar(blocf[:], blocf[:], nmbig[:, :1], None, OP.add)
                        nc.vector.tensor_copy(out=blocint_a[:, bsl], in_=blocf[:])
                    hsl = slice(st, st + w)
                    qint = ccp.tile([128, w], i32, name=f"qi{h}")
                    nc.vector.tensor_scalar(qint[:], gint_a[:, hsl], 7, None, OP.arith_shift_right)
                    nc.vector.tensor_scalar(segint_a[:, hsl], blocint_a[:, hsl], 7, None,
                                            OP.logical_shift_left)
                    nc.vector.tensor_tensor(out=segint_a[:, hsl], in0=segint_a[:, hsl],
                                            in1=qint[:], op=OP.add)
                    rint = ccp.tile([128, w], i32, name=f"ri{h}")
                    nc.vector.tensor_scalar(rint[:], gint_a[:, hsl], 127, None, OP.bitwise_and)
                    nc.vector.tensor_copy(out=rf_a[:, hsl], in_=rint[:])

                mgf_tiles = {}

                ohd_tiles = {}

                def emit_group_a1(g):
                    # one-hot scatter data + fragment gather
                    ohd = ocp.tile([128, 4, 128], f32)
                    ohd_tiles[g] = ohd
                    for j in range(4):
                        nc.vector.tensor_scalar(ohd[:, j, :], iota127[:],
                                                rf_a[:, 4 * g + j:4 * g + j + 1],
                                                None, OP.is_equal)
                    mgf = ocp.tile([128, 4, 2 * D], f32)
                    mgf_tiles[g] = mgf
                    nc.vector.memset(mgf[:, :, 0:D], 0.0)
                    for j in range(4):
                        nc.gpsimd.indirect_dma_start(
                            out=mgf[:, j, 0:D], out_offset=None, in_=frag[:],
                            in_offset=bass.IndirectOffsetOnAxis(
                                ap=gint_a[:, 4 * g + j:4 * g + j + 1], axis=0),
                            bounds_check=N - 1, oob_is_err=False)

                def emit_group_a2(g):
                    ohd = ohd_tiles.pop(g)
                    for j in range(4):
                        nc.gpsimd.indirect_dma_start(
                            out=onehot_v, out_offset=bass.IndirectOffsetOnAxis(
                                ap=segint_a[:, 4 * g + j:4 * g + j + 1], axis=0),
                            in_=ohd[:, j, :], in_offset=None,
                            bounds_check=BS * 128 - 1, oob_is_err=False)

                def emit_group_b1(g):
                    # merger = relu(cond @ Wa + fragment @ Wb + bm), bf16 matmuls
                    mgf = mgf_tiles[g]
                    ctb = ocp.tile([128, 4, D], bf16)
                    for j in range(4):
                        nc.gpsimd.dma_start(out=ctb[:, j, :],
                                            in_=cond[(4 * g + j) * 128:(4 * g + j + 1) * 128, :])
                    ftb = ocp.tile([128, 4, D], bf16)
                    nc.vector.tensor_copy(out=ftb[:], in_=mgf[:, :, 0:D])
                    for j in range(4):
                        pt2 = psh.tile([128, 256], bf16, name="ptc", tag="mgp", bufs=2)
                        for dc in range(2):
                            nc.tensor.transpose(pt2[:, dc * 128:(dc + 1) * 128],
                                                ctb[:, j, dc * 128:(dc + 1) * 128], identb[:])
                        cT = ocp.tile([128, 2, 128], bf16)
                        nc.scalar.copy(cT[:, 0, :], pt2[:, 0:128])
                        nc.scalar.copy(cT[:, 1, :], pt2[:, 128:256])
                        pt3 = psh.tile([128, 256], bf16, name="ptf", tag="mgp", bufs=2)
                        for dc in range(2):
                            nc.tensor.transpose(pt3[:, dc * 128:(dc + 1) * 128],
                                                ftb[:, j, dc * 128:(dc + 1) * 128], identb[:])
                        fT = ocp.tile([128, 2, 128], bf16)
                        nc.scalar.copy(fT[:, 0, :], pt3[:, 0:128])
                        nc.scalar.copy(fT[:, 1, :], pt3[:, 128:256])
                        pm = psh.tile([128, D], f32, name="pm", tag="mgp", bufs=2)
                        first = True
                        if with_bm:
                            nc.tensor.matmul(pm[:], ones1[:], bmsb[:], start=True, stop=False)
                            first = False
                        for dc in range(2):
                            nc.tensor.matmul(pm[:], cT[:, dc, :], wasb[:, dc, :],
                                             start=first and dc == 0, stop=False)
                        for dc in range(2):
                            nc.tensor.matmul(pm[:], fT[:, dc, :], wbsb[:, dc, :],
                                             start=False, stop=(dc == 1))
                        nc.scalar.activation(mgf[:, j, D:2 * D], pm[:], AF.Relu)

                def emit_group_b2(g):
                    mgf = mgf_tiles[g]
                    for j in range(4):
                        nc.gpsimd.indirect_dma_start(
                            out=frag_o[:], out_offset=bass.IndirectOffsetOnAxis(
                                ap=blocint_a[:, 4 * g + j:4 * g + j + 1], axis=0),
                            in_=mgf[:, j, 0:D], in_offset=None,
                            bounds_check=BS - 1, oob_is_err=False)

                def emit_group_b3(g):
                    mgf = mgf_tiles.pop(g)
                    for j in range(4):
                        nc.gpsimd.indirect_dma_start(
                            out=merg_o[:], out_offset=bass.IndirectOffsetOnAxis(
                                ap=blocint_a[:, 4 * g + j:4 * g + j + 1], axis=0),
                            in_=mgf[:, j, D:2 * D], in_offset=None,
                            bounds_check=BS - 1, oob_is_err=False)

                pso_tiles = [psh]  # reuse hT psum pool slots for merger-phase psums
                deferred = []
                for bt in range(NBT):
                    # condT for this B-tile: [128, 2, 512] (d-chunk, b)
                    ctile = btp.tile([128, 4, D], f32)
                    for j in range(4):
                        nc.sync.dma_start(
                            out=ctile[:, j, :],
                            in_=cond[(bt * 4 + j) * 128:(bt * 4 + j + 1) * 128, :])
                    condT = btp.tile([128, 2, 512], f32)
                    for dc in range(2):
                        pt = psh.tile([128, 512], f32, tag="ph", bufs=2)
                        for j in range(4):
                            nc.tensor.transpose(
                                pt[:, j * 128:(j + 1) * 128],
                                ctile[:, j, dc * 128:(dc + 1) * 128], ident[:])
                        nc.scalar.copy(condT[:, dc, :], pt[:])
                    # hT = relu(W1.T @ condT + b1): bf16 hi + residual lo
                    hT = btp.tile([128, 8, 512], bf16)
                    hTlo = btp.tile([128, 8, 512], bf16)
                    for hc in range(8):
                        ph = psh.tile([128, 512], f32, tag="ph", bufs=2)
                        for kc in range(2):
                            nc.tensor.matmul(ph[:], w1sb[:, kc, hc * 128:(hc + 1) * 128],
                                             condT[:, kc, :], start=(kc == 0), stop=(kc == 1))
                        nc.scalar.activation(hT[:, hc, :], ph[:], AF.Relu,
                                             bias=b1sb[:, hc:hc + 1])
                        # lo = relu(ph + b1) - hi
                        rl = bcp.tile([128, 512], f32)
                        nc.vector.tensor_scalar(rl[:], ph[:], b1sb[:, hc:hc + 1], 0.0,
                                                OP.add, OP.max)
                        nc.vector.tensor_tensor(out=hTlo[:, hc, :], in0=rl[:],
                                                in1=hT[:, hc, :], op=OP.subtract)
                    if bt == 0:
                        # W2 loads emitted after bt0's cond/hT DMAs so the pipeline ramps fast
                        for ns in range(4):
                            nsl = slice(ns * 512, (ns + 1) * 512)
                            for hc in range(8):
                                nc.sync.dma_start(out=w2hi[:, hc, nsl],
                                                  in_=w2h[hc * 128:(hc + 1) * 128, nsl])
                            for hc in range(8):
                                nc.sync.dma_start(out=w2lo[:, hc, nsl],
                                                  in_=w2l[hc * 128:(hc + 1) * 128, nsl])

                    for bc in range(4):
                        bg = bt * 4 + bc
                        zb = bcp.tile([128, NS], f32)
                        for ns in range(4):
                            nsl = slice(ns * 512, (ns + 1) * 512)
                            gum = lop.tile([128, 512], f32)
                            nc.sync.dma_start(out=gum[:], in_=gumb[bg * 128:(bg + 1) * 128, nsl])
                            msk = lop.tile([128, 512], u8)
                            nc.sync.dma_start(out=msk[:], in_=maskb[bg * 128:(bg + 1) * 128, nsl])
                            mbig = lop.tile([128, 512], f32)
                            nc.scalar.activation(mbig[:], msk[:], AF.Copy, scale=BIGNEG)
                            pl = psp.tile([128, 512], f32)
                            first = True
                            if with_b2:
                                nc.tensor.matmul(pl[:], ones1[:], b2sb[:, nsl],
                                                 start=True, stop=False)
                                first = False
                            for hc in range(8):
                                nc.tensor.matmul(
                                    pl[:], hT[:, hc, bc * 128:(bc + 1) * 128],
                                    w2hi[:, hc, nsl],
                                    start=first and hc == 0, stop=False)
                            for hc in range(8):
                                nc.tensor.matmul(
                                    pl[:], hT[:, hc, bc * 128:(bc + 1) * 128],
                                    w2lo[:, hc, nsl], start=False, stop=False)
                            for hc in range(8):
                                nc.tensor.matmul(
                                    pl[:], hTlo[:, hc, bc * 128:(bc + 1) * 128],
                                    w2hi[:, hc, nsl], start=False, stop=(hc == 7))
                            # logits_m = (psum + mbig) + mbig  (exact -inf on masked);
                            # z only needs the single-add (-3.4e38) masking.
                            t0 = lop.tile([128, 512], f32)
                            nc.scalar.copy(t0[:], pl[:])
                            t1 = lop.tile([128, 512], f32)
                            nc.gpsimd.tensor_tensor(out=t1[:], in0=t0[:],
                                                    in1=mbig[:], op=OP.add)
                            lm = lop.tile([128, 512], f32)
                            nc.gpsimd.tensor_tensor(out=lm[:], in0=t1[:],
                                                    in1=mbig[:], op=OP.add)
                            nc.sync.dma_start(out=logits_o[bg * 128:(bg + 1) * 128, nsl],
                                              in_=lm[:])
                            # z = logits_m + gumbel (argmax input; -3.4e38 on masked)
                            nc.vector.tensor_tensor(out=zb[:, nsl], in0=t1[:],
                                                    in1=gum[:], op=OP.add)
                        mx8 = bcp.tile([128, 8], f32)
                        nc.vector.max(mx8[:], zb[:])
                        ix8 = bcp.tile([128, 8], u32)
                        nc.vector.max_index(ix8[:], mx8[:], zb[:])
                        nc.vector.tensor_copy(out=lboth[:, bg:bg + 1], in_=mx8[:, 0:1])
                        nc.vector.tensor_copy(out=lboth[:, 32 + bg:33 + bg], in_=ix8[:, 0:1])
                        # interleave deferred output units (finer grained, 2/row-chunk)
                        if deferred:
                            deferred.pop(0)()
                        if deferred:
                            deferred.pop(0)()
                    for s, (st, w) in enumerate(SPLITS):
                        if (bt + 1) * 4 == st + w and phase >= 2:
                            emit_pack(s)
                            deferred += [lambda s=s: (emit_combine(s), emit_prep(s))]
                            if phase >= 3:
                                for g in range(st // 4, (st + w) // 4):
                                    deferred += [lambda g=g: emit_group_a1(g),
                                                 lambda g=g: emit_group_a2(g),
                                                 lambda g=g: emit_group_b1(g),
                                                 lambda g=g: emit_group_b2(g),
                                                 lambda g=g: emit_group_b3(g)]

                # ---- tail: remaining deferred units (quarter 3)
                if phase >= 2:
                    for fn in deferred:
                        fn()
                if phase < 2:
                    nc.sync.dma_start(out=frag_o[0:128, 0:64], in_=lboth[:])
    nc.finalize()
    return nc


def _get(mode, with_b2, with_bm):
    key = (mode, with_b2, with_bm)
    if key not in _CACHE:
        _CACHE[key] = _build(mode, with_b2, with_bm)
    return _CACHE[key]


def kernel(condition, mask, gumbel, frag_table, W1, b1, W2, b2, Wa, Wb, bm,
           _trace=False, _tmpdir=None):
    condition = np.ascontiguousarray(condition, np.float32)
    mask_u8 = np.ascontiguousarray(mask).view(np.uint8)
    gumbel = np.ascontiguousarray(gumbel, np.float32)
    frag_table = np.ascontiguousarray(frag_table, np.float32)
    with_b2 = bool(np.any(b2))
    with_bm = bool(np.any(bm))
    nc = _get(MODE, with_b2, with_bm)

    b1t = np.ascontiguousarray(np.asarray(b1, np.float32).reshape(H // 128, 128).T)
    import ml_dtypes
    w2f = np.asarray(W2, np.float32)
    w2h_full = w2f.astype(ml_dtypes.bfloat16)
    w2l_full = (w2f - w2h_full.astype(np.float32)).astype(ml_dtypes.bfloat16)
    in_maps = []
    for c in range(NC):
        nsl = slice(c * NS, (c + 1) * NS)
        in_maps.append(dict(
            cond=condition,
            maskb=np.ascontiguousarray(mask_u8[:, nsl]),
            gumb=np.ascontiguousarray(gumbel[:, nsl]),
            w1=np.ascontiguousarray(W1, np.float32),
            b1=b1t,
            w2h=w2h_full[:, nsl].copy(),
            w2l=w2l_full[:, nsl].copy(),
            b2=np.ascontiguousarray(np.asarray(b2, np.float32)[nsl]).reshape(1, NS),
            frag=frag_table,
            wa=np.ascontiguousarray(Wa, np.float32),
            wb=np.ascontiguousarray(Wb, np.float32),
            bm=np.ascontiguousarray(np.asarray(bm, np.float32)).reshape(1, D),
        ))

    res = run_bass_kernel_spmd(nc, in_maps, core_ids=list(range(NC)), trace=_trace,
                               tmpdir=_tmpdir)
    rs = res.results
    logits = np.concatenate([rs[c]["logits_o"] for c in range(NC)], axis=1)
    onehot = np.concatenate([rs[c]["onehot_o"] for c in range(NC)], axis=0)
    fragment = np.concatenate([rs[c]["frag_o"] for c in range(NC)], axis=0)
    merger = np.concatenate([rs[c]["merg_o"] for c in range(NC)], axis=0)
    if _trace:
        kernel.last_exec_time_ns = res.exec_time_ns
    return logits, onehot, fragment, merger


# revision 31
# speedup vs baseline: 1.0732x; 1.0732x over previous
"""Trainium2 Bass kernel for the Actor moe_routing module.

Computes, on 8 NeuronCores (tensor-parallel over the fragment dim N):
    h        = relu(cond @ W1 + b1)                      [B, H]
    logits   = where(mask, -inf, h @ W2 + b2)            [B, N]
    idx      = argmax(logits + gumbel, axis=1)           [B]
    onehot   = one_hot(idx, N)                           [B, N]  (== gumbel-softmax hard forward)
    fragment = frag_table[idx]                           [B, D]
    merger   = relu(cond @ Wa + fragment @ Wb + bm)      [B, D]

Sharding: each core owns a 2048-column shard of N (W2/b2/mask/gumbel column
shards, logits written per-shard); the argmax is combined across cores with a
tiny AllGather; onehot/fragment/merger are produced B-sharded (512 rows per
core) using indirect DMA with out-of-bounds predication.

MODE:
  "fp32" - full-precision matmuls for h/logits (4 cyc/row).
  "bf3"  - 3-term bf16 split (hi*hi + hi*lo + lo*hi) for the logits matmul
           (3 cyc/row, ~2^-17 rel err; verified 0 argmax flips on the
           reference inputs).
"""
import sys
sys.path.insert(0, '/opt/trn_rl_repo')

import numpy as np
import concourse.bass as bass
import concourse.mybir as mybir
import concourse.tile as tile
from concourse import bacc
from concourse.bass_utils import run_bass_kernel_spmd
from concourse.masks import make_identity

f32 = mybir.dt.float32
bf16 = mybir.dt.bfloat16
i32 = mybir.dt.int32
u32 = mybir.dt.uint32
u8 = mybir.dt.uint8
OP = mybir.AluOpType
AF = mybir.ActivationFunctionType

B, N, D, H = 4096, 16384, 256, 1024
NC = 8
NS = N // NC          # 2048 n-columns per core
BS = B // NC          # 512 b-rows per core (output sharding)
NBT = 8               # B-tiles of 512 rows
NBC = B // 128        # 32 row-chunks of 128
BIGNEG = -3.4e38      # x + BIGNEG + BIGNEG == -inf for |x| small
OOB = 1 << 22

MODE = "bf3"

_CACHE = {}


def _build(mode, with_b2, with_bm):
    import os
    phase = int(os.environ.get("KPH", "3"))
    nc = bacc.Bacc("TRN2", target_bir_lowering=False, debug=False, num_devices=NC)

    cond = nc.dram_tensor("cond", [B, D], f32, kind="ExternalInput").ap()
    maskb = nc.dram_tensor("maskb", [B, NS], u8, kind="ExternalInput").ap()
    gumb = nc.dram_tensor("gumb", [B, NS], f32, kind="ExternalInput").ap()
    w1 = nc.dram_tensor("w1", [D, H], f32, kind="ExternalInput").ap()
    b1 = nc.dram_tensor("b1", [128, H // 128], f32, kind="ExternalInput").ap()
    w2h = nc.dram_tensor("w2h", [H, NS], bf16, kind="ExternalInput").ap()
    w2l = nc.dram_tensor("w2l", [H, NS], bf16, kind="ExternalInput").ap()
    b2 = nc.dram_tensor("b2", [1, NS], f32, kind="ExternalInput").ap()
    frag = nc.dram_tensor("frag", [N, D], f32, kind="ExternalInput").ap()
    wa = nc.dram_tensor("wa", [D, D], f32, kind="ExternalInput").ap()
    wb = nc.dram_tensor("wb", [D, D], f32, kind="ExternalInput").ap()
    bm = nc.dram_tensor("bm", [1, D], f32, kind="ExternalInput").ap()

    logits_o = nc.dram_tensor("logits_o", [B, NS], f32, kind="ExternalOutput").ap()
    onehot_o = nc.dram_tensor("onehot_o", [BS, N], f32, kind="ExternalOutput").ap()
    frag_o = nc.dram_tensor("frag_o", [BS, D], f32, kind="ExternalOutput").ap()
    merg_o = nc.dram_tensor("merg_o", [BS, D], f32, kind="ExternalOutput").ap()


    with tile.TileContext(nc) as tc:
        with tc.tile_pool(name="const", bufs=1) as cp:
            ident = cp.tile([128, 128], f32)
            make_identity(nc, ident[:])
            identb = cp.tile([128, 128], bf16)
            nc.vector.tensor_copy(out=identb[:], in_=ident[:])
            ones1 = cp.tile([1, 128], f32)
            nc.gpsimd.memset(ones1[:], 1.0)
            # iota127[p, j] = j  (compare target for one-hot remainders)
            iota127i = cp.tile([128, 128], i32)
            nc.gpsimd.iota(iota127i[:], pattern=[[1, 128]], base=0, channel_multiplier=0)
            iota127 = cp.tile([128, 128], f32)
            nc.vector.tensor_copy(out=iota127[:], in_=iota127i[:])
            # biota[p, c] = 128*c + p  (global row index per (chunk, partition))
            biotai = cp.tile([128, NBC], i32)
            nc.gpsimd.iota(biotai[:], pattern=[[128, NBC]], base=0, channel_multiplier=1)
            # local (max | argmax) for all 32 row-chunks, AllGather payload layout
            lboth = cp.tile([128, 64], f32)
            b1sb = cp.tile([128, H // 128], f32)
            nc.sync.dma_start(out=b1sb[:], in_=b1[:])
            b2sb = cp.tile([1, NS], f32)
            nc.sync.dma_start(out=b2sb[:], in_=b2[:])
            bmsb = cp.tile([1, D], f32)
            nc.sync.dma_start(out=bmsb[:], in_=bm[:])
            # partition id -> f32 broadcast column
            pid_sb = cp.tile([1, 1], u32)
            nc.sync.dma_start(out=pid_sb[:], in_=nc.partition_id_tensor.ap()[:])
            pid1f = cp.tile([1, 1], f32)
            nc.vector.tensor_copy(out=pid1f[:], in_=pid_sb[:])
            pidc = cp.tile([128, 1], f32)

            with tc.tile_pool(name="psc", bufs=1, space="PSUM") as psc:
                pp = psc.tile([128, 1], f32)
                nc.tensor.matmul(pp[:], ones1[:], pid1f[:], start=True, stop=True)
                nc.scalar.copy(pidc[:], pp[:])

            # W1 resident [128, 2, H] (k-chunk major)
            w1sb = cp.tile([128, 2, H], f32)
            for kc in range(2):
                nc.sync.dma_start(out=w1sb[:, kc, :], in_=w1[kc * 128:(kc + 1) * 128, :])
            wasb = cp.tile([128, 2, D], bf16)
            wbsb = cp.tile([128, 2, D], bf16)
            wtmp = cp.tile([128, 2, D], f32)
            for kc in range(2):
                nc.sync.dma_start(out=wtmp[:, kc, :], in_=wa[kc * 128:(kc + 1) * 128, :])
            nc.vector.tensor_copy(out=wasb[:], in_=wtmp[:])
            wtmp2 = cp.tile([128, 2, D], f32)
            for kc in range(2):
                nc.sync.dma_start(out=wtmp2[:, kc, :], in_=wb[kc * 128:(kc + 1) * 128, :])
            nc.vector.tensor_copy(out=wbsb[:], in_=wtmp2[:])

            # ---------------- main loop + split collectives ----------------
            # B is processed in two halves; each half's argmax AllGather, combine
            # and output phase interleave under the other half's matmul stream.
            with tc.tile_pool(name="w2p", bufs=1) as w2p, \
                 tc.tile_pool(name="cc", bufs=1) as ccp, \
                 tc.tile_pool(name="ccd", bufs=1, space="DRAM") as ccd, \
                 tc.tile_pool(name="oc", bufs=1) as ocp, \
                 tc.tile_pool(name="bt", bufs=2) as btp, \
                 tc.tile_pool(name="bc", bufs=2) as bcp, \
                 tc.tile_pool(name="lo", bufs=2) as lop, \
                 tc.tile_pool(name="ps", bufs=2, space="PSUM") as psp, \
                 tc.tile_pool(name="psh", bufs=2, space="PSUM") as psh:
                if mode == "fp32":
                    raise NotImplementedError("fp32 mode disabled after host-side W2 split")
                w2hi = w2p.tile([128, 8, NS], bf16)
                w2lo = w2p.tile([128, 8, NS], bf16)
                SPLITS = [(0, 12), (12, 12), (24, 4), (28, 4)]
                cc_in = [ccd.tile([128, 2 * w], f32, name=f"cci{h}")
                         for h, (_, w) in enumerate(SPLITS)]
                cc_out = [ccd.tile([NC * 128, 2 * w], f32, addr_space="Shared", name=f"cco{h}")
                          for h, (_, w) in enumerate(SPLITS)]
                gidx = ccp.tile([128, NBC], f32)
                gint_a = ccp.tile([128, NBC], i32)
                blocint_a = ccp.tile([128, NBC], i32)
                segint_a = ccp.tile([128, NBC], i32)
                rf_a = ccp.tile([128, NBC], f32)
                pid512 = ccp.tile([128, 1], f32)
                nc.vector.tensor_scalar(pid512[:], pidc[:], float(BS), None, OP.mult)
                onehot_v = onehot_o.rearrange("b (q r) -> (b q) r", r=128)

                def emit_pack(h):
                    st, w = SPLITS[h]
                    # payload [128, 2w]: (lmax cols | lidx cols) for split h
                    nc.gpsimd.dma_start(out=cc_in[h][:, 0:w], in_=lboth[:, st:st + w])
                    nc.gpsimd.dma_start(out=cc_in[h][:, w:2 * w],
                                        in_=lboth[:, 32 + st:32 + st + w])
                    nc.gpsimd.collective_compute(
                        "AllGather", OP.bypass,
                        replica_groups=[list(range(NC))],
                        ins=[cc_in[h].opt()], outs=[cc_out[h].opt()],
                    )

                def emit_combine(h):
                    st, w = SPLITS[h]
                    bsl = slice(st, st + w)
                    valsidx = ccp.tile([128, NC, 2 * w], f32, name=f"vi{h}")
                    for r in range(NC):
                        nc.sync.dma_start(out=valsidx[:, r, :],
                                          in_=cc_out[h][r * 128:(r + 1) * 128, :])
                    gmax = ccp.tile([128, w], f32, name=f"gmax{h}")
                    nc.vector.tensor_copy(out=gmax[:], in_=valsidx[:, 0, 0:w])
                    for r in range(1, NC):
                        nc.vector.tensor_tensor(out=gmax[:], in0=gmax[:],
                                                in1=valsidx[:, r, 0:w], op=OP.max)
                    taken = ccp.tile([128, w], f32, name=f"tk{h}")
                    nc.vector.memset(taken[:], 0.0)
                    nc.vector.memset(gidx[:, bsl], 0.0)
                    wt = ccp.tile([128, w], f32, name=f"wt{h}")
                    omt = ccp.tile([128, w], f32, name=f"omt{h}")
                    term = ccp.tile([128, w], f32, name=f"tm{h}")
                    for r in range(NC):
                        nc.vector.tensor_tensor(out=wt[:], in0=valsidx[:, r, 0:w],
                                                in1=gmax[:], op=OP.is_equal)
                        nc.vector.tensor_scalar(omt[:], taken[:], -1.0, 1.0, OP.mult, OP.add)
                        nc.vector.tensor_tensor(out=wt[:], in0=wt[:], in1=omt[:], op=OP.mult)
                        nc.vector.tensor_scalar(term[:], valsidx[:, r, w:2 * w], float(r * NS),
                                                None, OP.add)
                        nc.vector.tensor_tensor(out=term[:], in0=term[:], in1=wt[:], op=OP.mult)
                        nc.vector.tensor_tensor(out=gidx[:, bsl], in0=gidx[:, bsl],
                                                in1=term[:], op=OP.add)
                        nc.vector.tensor_tensor(out=taken[:], in0=taken[:], in1=wt[:], op=OP.max)

                def emit_prep(h):
                    st, w = SPLITS[h]
                    # per-group index prep for split h (all DVE)
                    for g in range(st // 4, (st + w) // 4):
                        bsl = slice(4 * g, 4 * g + 4)
                        mine = ccp.tile([128, 1], f32, name=f"mine{g}")
                        nc.vector.tensor_scalar(mine[:], pidc[:], float(g), None, OP.is_equal)
                        nmbig = ccp.tile([128, 1], f32, name=f"nmbig{g}")
                        nc.vector.tensor_scalar(nmbig[:], mine[:], float(-OOB), float(OOB),
                                                OP.mult, OP.add)
                        gadj = ccp.tile([128, 4], f32, name=f"gadj{g}")
                        nc.vector.tensor_scalar(gadj[:], gidx[:, bsl], nmbig[:, :1], None, OP.add)
                        nc.vector.tensor_copy(out=gint_a[:, bsl], in_=gadj[:])
                        blocf = ccp.tile([128, 4], f32, name=f"blocf{g}")
                        nc.vector.tensor_copy(out=blocf[:], in_=biotai[:, bsl])
                        nc.vector.tensor_scalar(blocf[:], blocf[:], pid512[:, :1], None, OP.subtract)
                        nc.vector.tensor_scalar(blocf[:], blocf[:], nmbig[:, :1], None, OP.add)
                        nc.vector.tensor_copy(out=blocint_a[:, bsl], in_=blocf[:])
                    hsl = slice(st, st + w)
                    qint = ccp.tile([128, w], i32, name=f"qi{h}")
                    nc.vector.tensor_scalar(qint[:], gint_a[:, hsl], 7, None, OP.arith_shift_right)
                    nc.vector.tensor_scalar(segint_a[:, hsl], blocint_a[:, hsl], 7, None,
                                            OP.logical_shift_left)
                    nc.vector.tensor_tensor(out=segint_a[:, hsl], in0=segint_a[:, hsl],
                                            in1=qint[:], op=OP.add)
                    rint = ccp.tile([128, w], i32, name=f"ri{h}")
                    nc.vector.tensor_scalar(rint[:], gint_a[:, hsl], 127, None, OP.bitwise_and)
                    nc.vector.tensor_copy(out=rf_a[:, hsl], in_=rint[:])

                mgf_tiles = {}

                ohd_tiles = {}

                def emit_group_a1(g):
                    # one-hot scatter data + fragment gather
                    ohd = ocp.tile([128, 4, 128], f32)
                    ohd_tiles[g] = ohd
                    for j in range(4):
                        nc.vector.tensor_scalar(ohd[:, j, :], iota127[:],
                                                rf_a[:, 4 * g + j:4 * g + j + 1],
                                                None, OP.is_equal)
                    mgf = ocp.tile([128, 4, 2 * D], f32)
                    mgf_tiles[g] = mgf
                    nc.vector.memset(mgf[:, :, 0:D], 0.0)
                    for j in range(4):
                        nc.gpsimd.indirect_dma_start(
                            out=mgf[:, j, 0:D], out_offset=None, in_=frag[:],
                            in_offset=bass.IndirectOffsetOnAxis(
                                ap=gint_a[:, 4 * g + j:4 * g + j + 1], axis=0),
                            bounds_check=N - 1, oob_is_err=False)

                def emit_group_a2(g):
                    ohd = ohd_tiles.pop(g)
                    for j in range(4):
                        nc.gpsimd.indirect_dma_start(
                            out=onehot_v, out_offset=bass.IndirectOffsetOnAxis(
                                ap=segint_a[:, 4 * g + j:4 * g + j + 1], axis=0),
                            in_=ohd[:, j, :], in_offset=None,
                            bounds_check=BS * 128 - 1, oob_is_err=False)

                def emit_group_b1(g):
                    # merger = relu(cond @ Wa + fragment @ Wb + bm), bf16 matmuls
                    mgf = mgf_tiles[g]
                    ctb = ocp.tile([128, 4, D], bf16)
                    for j in range(4):
                        nc.gpsimd.dma_start(out=ctb[:, j, :],
                                            in_=cond[(4 * g + j) * 128:(4 * g + j + 1) * 128, :])
                    ftb = ocp.tile([128, 4, D], bf16)
                    nc.vector.tensor_copy(out=ftb[:], in_=mgf[:, :, 0:D])
                    for j in range(4):
                        pt2 = psh.tile([128, 256], bf16, name="ptc", tag="mgp", bufs=2)
                        for dc in range(2):
                            nc.tensor.transpose(pt2[:, dc * 128:(dc + 1) * 128],
                                                ctb[:, j, dc * 128:(dc + 1) * 128], identb[:])
                        cT = ocp.tile([128, 2, 128], bf16)
                        nc.scalar.copy(cT[:, 0, :], pt2[:, 0:128])
                        nc.scalar.copy(cT[:, 1, :], pt2[:, 128:256])
                        pt3 = psh.tile([128, 256], bf16, name="ptf", tag="mgp", bufs=2)
                        for dc in range(2):
                            nc.tensor.transpose(pt3[:, dc * 128:(dc + 1) * 128],
                                                ftb[:, j, dc * 128:(dc + 1) * 128], identb[:])
                        fT = ocp.tile([128, 2, 128], bf16)
                        nc.scalar.copy(fT[:, 0, :], pt3[:, 0:128])
                        nc.scalar.copy(fT[:, 1, :], pt3[:, 128:256])
                        pm = psh.tile([128, D], f32, name="pm", tag="mgp", bufs=2)
                        first = True
                        if with_bm:
                            nc.tensor.matmul(pm[:], ones1[:], bmsb[:], start=True, stop=False)
                            first = False
                        for dc in range(2):
                            nc.tensor.matmul(pm[:], cT[:, dc, :], wasb[:, dc, :],
                                             start=first and dc == 0, stop=False)
                        for dc in range(2):
                            nc.tensor.matmul(pm[:], fT[:, dc, :], wbsb[:, dc, :],
                                             start=False, stop=(dc == 1))
                        nc.scalar.activation(mgf[:, j, D:2 * D], pm[:], AF.Relu)

                def emit_group_b2(g):
                    mgf = mgf_tiles[g]
                    for j in range(4):
                        nc.gpsimd.indirect_dma_start(
                            out=frag_o[:], out_offset=bass.IndirectOffsetOnAxis(
                                ap=blocint_a[:, 4 * g + j:4 * g + j + 1], axis=0),
                            in_=mgf[:, j, 0:D], in_offset=None,
                            bounds_check=BS - 1, oob_is_err=False)

                def emit_group_b3(g):
                    mgf = mgf_tiles.pop(g)
                    for j in range(4):
                        nc.gpsimd.indirect_dma_start(
                            out=merg_o[:], out_offset=bass.IndirectOffsetOnAxis(
                                ap=blocint_a[:, 4 * g + j:4 * g + j + 1], axis=0),
                            in_=mgf[:, j, D:2 * D], in_offset=None,
                            bounds_check=BS - 1, oob_is_err=False)

                pso_tiles = [psh]  # reuse hT psum pool slots for merger-phase psums
                deferred = []
                for bt in range(NBT):
                    # condT for this B-tile: [128, 2, 512] (d-chunk, b)
                    ctile = btp.tile([128, 4, D], f32)
                    for j in range(4):
                        nc.sync.dma_start(
                            out=ctile[:, j, :],
                            in_=cond[(bt * 4 + j) * 128:(bt * 4 + j + 1) * 128, :])
                    condT = btp.tile([128, 2, 512], f32)
                    for dc in range(2):
                        pt = psh.tile([128, 512], f32, tag="ph", bufs=2)
                        for j in range(4):
                            nc.tensor.transpose(
                                pt[:, j * 128:(j + 1) * 128],
                                ctile[:, j, dc * 128:(dc + 1) * 128], ident[:])
                        nc.scalar.copy(condT[:, dc, :], pt[:])
                    # hT = relu(W1.T @ condT + b1): bf16 hi + residual lo
                    hT = btp.tile([128, 8, 512], bf16)
                    hTlo = btp.tile([128, 8, 512], bf16)
                    for hc in range(8):
                        ph = psh.tile([128, 512], f32, tag="ph", bufs=2)
                        for kc in range(2):
                            nc.tensor.matmul(ph[:], w1sb[:, kc, hc * 128:(hc + 1) * 128],
                                             condT[:, kc, :], start=(kc == 0), stop=(kc == 1))
                        nc.scalar.activation(hT[:, hc, :], ph[:], AF.Relu,
                                             bias=b1sb[:, hc:hc + 1])
                        # lo = relu(ph + b1) - hi
                        rl = bcp.tile([128, 512], f32)
                        nc.vector.tensor_scalar(rl[:], ph[:], b1sb[:, hc:hc + 1], 0.0,
                                                OP.add, OP.max)
                        nc.vector.tensor_tensor(out=hTlo[:, hc, :], in0=rl[:],
                                                in1=hT[:, hc, :], op=OP.subtract)
                    if bt == 0:
                        # W2 loads emitted after bt0's cond/hT DMAs so the pipeline ramps fast
                        for ns in range(4):
                            nsl = slice(ns * 512, (ns + 1) * 512)
                            for hc in range(8):
                                nc.sync.dma_start(out=w2hi[:, hc, nsl],
                                                  in_=w2h[hc * 128:(hc + 1) * 128, nsl])
                            for hc in range(8):
                                nc.sync.dma_start(out=w2lo[:, hc, nsl],
                                                  in_=w2l[hc * 128:(hc + 1) * 128, nsl])

                    for bc in range(4):
                        bg = bt * 4 + bc
                        zb = bcp.tile([128, NS], f32)
                        for ns in range(4):
                            nsl = slice(ns * 512, (ns + 1) * 512)
                            gum = lop.tile([128, 512], f32)
                            nc.sync.dma_start(out=gum[:], in_=gumb[bg * 128:(bg + 1) * 128, nsl])
                            msk = lop.tile([128, 512], u8)
                            nc.sync.dma_start(out=msk[:], in_=maskb[bg * 128:(bg + 1) * 128, nsl])
                            mbig = lop.tile([128, 512], f32)
                            nc.scalar.activation(mbig[:], msk[:], AF.Copy, scale=BIGNEG)
                            pl = psp.tile([128, 512], f32)
                            first = True
                            if with_b2:
                                nc.tensor.matmul(pl[:], ones1[:], b2sb[:, nsl],
                                                 start=True, stop=False)
                                first = False
                            for hc in range(8):
                                nc.tensor.matmul(
                                    pl[:], hT[:, hc, bc * 128:(bc + 1) * 128],
                                    w2hi[:, hc, nsl],
                                    start=first and hc == 0, stop=False)
                            for hc in range(8):
                                nc.tensor.matmul(
                                    pl[:], hT[:, hc, bc * 128:(bc + 1) * 128],
                                    w2lo[:, hc, nsl], start=False, stop=False)
                            for hc in range(8):
                                nc.tensor.matmul(
                                    pl[:], hTlo[:, hc, bc * 128:(bc + 1) * 128],
                                    w2hi[:, hc, nsl], start=False, stop=(hc == 7))
                            # logits_m = (psum + mbig) + mbig  (exact -inf on masked);
                            # z only needs the single-add (-3.4e38) masking.
                            t0 = lop.tile([128, 512], f32)
                            nc.scalar.copy(t0[:], pl[:])
                            t1 = lop.tile([128, 512], f32)
                            nc.gpsimd.tensor_tensor(out=t1[:], in0=t0[:],
                                                    in1=mbig[:], op=OP.add)
                            lm = lop.tile([128, 512], f32)
                            nc.gpsimd.tensor_tensor(out=lm[:], in0=t1[:],
                                                    in1=mbig[:], op=OP.add)
                            nc.sync.dma_start(out=logits_o[bg * 128:(bg + 1) * 128, nsl],
                                              in_=lm[:])
                            # z = logits_m + gumbel (argmax input; -3.4e38 on masked)
                            nc.vector.tensor_tensor(out=zb[:, nsl], in0=t1[:],
                                                    in1=gum[:], op=OP.add)
                        mx8 = bcp.tile([128, 8], f32)
                        nc.vector.max(mx8[:], zb[:])
                        ix8 = bcp.tile([128, 8], u32)
                        nc.vector.max_index(ix8[:], mx8[:], zb[:])
                        nc.vector.tensor_copy(out=lboth[:, bg:bg + 1], in_=mx8[:, 0:1])
                        nc.vector.tensor_copy(out=lboth[:, 32 + bg:33 + bg], in_=ix8[:, 0:1])
                        # interleave deferred output units (finer grained, 2/row-chunk)
                        if deferred:
                            deferred.pop(0)()
                        if deferred:
                            deferred.pop(0)()
                    for s, (st, w) in enumerate(SPLITS):
                        if (bt + 1) * 4 == st + w and phase >= 2:
                            emit_pack(s)
                            deferred += [lambda s=s: (emit_combine(s), emit_prep(s))]
                            if phase >= 3:
                                for g in range(st // 4, (st + w) // 4):
                                    deferred += [lambda g=g: emit_group_a1(g),
                                                 lambda g=g: emit_group_a2(g),
                                                 lambda g=g: emit_group_b1(g),
                                                 lambda g=g: emit_group_b2(g),
                                                 lambda g=g: emit_group_b3(g)]

                # ---- tail: remaining deferred units (quarter 3)
                if phase >= 2:
                    for fn in deferred:
                        fn()
                if phase < 2:
                    nc.sync.dma_start(out=frag_o[0:128, 0:64], in_=lboth[:])
    nc.finalize()
    return nc


def _get(mode, with_b2, with_bm):
    key = (mode, with_b2, with_bm)
    if key not in _CACHE:
        _CACHE[key] = _build(mode, with_b2, with_bm)
    return _CACHE[key]


def kernel(condition, mask, gumbel, frag_table, W1, b1, W2, b2, Wa, Wb, bm,
           _trace=False, _tmpdir=None):
    condition = np.ascontiguousarray(condition, np.float32)
    mask_u8 = np.ascontiguousarray(mask).view(np.uint8)
    gumbel = np.ascontiguousarray(gumbel, np.float32)
    frag_table = np.ascontiguousarray(frag_table, np.float32)
    with_b2 = bool(np.any(b2))
    with_bm = bool(np.any(bm))
    nc = _get(MODE, with_b2, with_bm)

    b1t = np.ascontiguousarray(np.asarray(b1, np.float32).reshape(H // 128, 128).T)
    import ml_dtypes
    w2f = np.asarray(W2, np.float32)
    w2h_full = w2f.astype(ml_dtypes.bfloat16)
    w2l_full = (w2f - w2h_full.astype(np.float32)).astype(ml_dtypes.bfloat16)
    in_maps = []
    for c in range(NC):
        nsl = slice(c * NS, (c + 1) * NS)
        in_maps.append(dict(
            cond=condition,
            maskb=np.ascontiguousarray(mask_u8[:, nsl]),
            gumb=np.ascontiguousarray(gumbel[:, nsl]),
            w1=np.ascontiguousarray(W1, np.float32),
            b1=b1t,
            w2h=w2h_full[:, nsl].copy(),
            w2l=w2l_full[:, nsl].copy(),
            b2=np.ascontiguousarray(np.asarray(b2, np.float32)[nsl]).reshape(1, NS),
            frag=frag_table,
            wa=np.ascontiguousarray(Wa, np.float32),
            wb=np.ascontiguousarray(Wb, np.float32),
            bm=np.ascontiguousarray(np.asarray(bm, np.float32)).reshape(1, D),
        ))

    res = run_bass_kernel_spmd(nc, in_maps, core_ids=list(range(NC)), trace=_trace,
                               tmpdir=_tmpdir)
    rs = res.results
    logits = np.concatenate([rs[c]["logits_o"] for c in range(NC)], axis=1)
    onehot = np.concatenate([rs[c]["onehot_o"] for c in range(NC)], axis=0)
    fragment = np.concatenate([rs[c]["frag_o"] for c in range(NC)], axis=0)
    merger = np.concatenate([rs[c]["merg_o"] for c in range(NC)], axis=0)
    if _trace:
        kernel.last_exec_time_ns = res.exec_time_ns
    return logits, onehot, fragment, merger
